# revision 58
# baseline (speedup 1.0000x reference)
"""Aitchison multi-head attention on 8 trn2 NeuronCores.

Per core: batch c//4, 4 heads (feature slice of 256). CLR centering is
linear -> folded into Wq/Wk + biases on host (fp64). Host sums the 4
partial output projections per batch and adds bo.

Steady state is PE-streaming-bound (~163us busy: scores 27 + PV 55 +
q/k/v proj 41 + Wo 14 + overheads, all bf16); the ACT exp train (128x
[128,1024] exps, ~1.0us each pipelined) runs just under it. Design:
- Host pre-reshapes all tensors so every load is a simple 3D AP slice;
  input DMAs are tq-block pieces issued on both HW-DGE queues
  (sync + scalar) in consumer-deadline order; the link runs ~310GB/s,
  so arrival order == issue order. First exp fires ~28us.
- A PE warmup burst (garbage matmuls into ps_a) at t=0 plus warmups
  paced between the DMA-gated lead-in projection matmuls hold the HAM
  clock gate at 2.4GHz through the lead-in (in-order PE queue: real
  work is never queued behind a stalled load).
- scores: per group the 2 heads' matmuls run concurrently via
  row-tiling (lhsT base partitions 0/64 -> tile_position auto-derive).
- PV is bf16 (fp8 measured at rel_err 2.4e-2 > the 2e-2 gate — fp8's
  relative error lands on the dominant softmax weights; a linear int8
  quant would be fine but TRN2 bass has no integer matmul). The
  softmax denominator is free: a ones-column in v makes PSUM row 64
  the Z row.
- PV chunks pop lag-1 behind their exp group from a FIFO (cap 2/slot);
  the 1/Z chain is reciprocal_approx_fast + gpsimd broadcast + DVE
  mult; the last unit uses a latency-optimized chain (pv copies on the
  idle scalar engine, broadcast as one PE sel-matrix outer product).
- k/q/v projection blocks and Wo half-groups (~0.5us filler grains)
  are spread across units by deadline; the tail keeps the PE warm with
  dummy matmuls while the final normalize chain runs, then drains the
  last Wo block with casts split across scalar+vector and per-half
  DMAs.
"""
import sys
import types

sys.path.insert(0, "/opt/trn_rl_repo")

from collections import deque

import numpy as np
import ml_dtypes

import concourse.bass as bass
import concourse.tile as tile
from concourse import bacc, mybir
from concourse.bass_utils import run_bass_kernel_spmd

B, T, E, H, Dh = 2, 2048, 1024, 16, 64
NCORES = 8
HPC = 4            # heads per core
F = HPC * Dh       # 256 features per core
SCALE = 8.0        # sqrt(Dh)
KC = E // 128      # 8 e-chunks in projections
BF = mybir.dt.bfloat16
F32 = mybir.dt.float32
FP8 = mybir.dt.float8e4
BF_NP = ml_dtypes.bfloat16

FP8_PV = False     # fp8 PV measured at rel_err 2.4e-2 (> 2e-2 gate): any
                   # fp8 in the attention path costs ~2e-2 broad noise.


def _install_ntff_hook():
    """trace=True under axon needs antenv.axon_hooks, missing in this image."""
    if "antenv.axon_hooks" in sys.modules:
        return
    try:
        from trn_agent_boot.trn_boot import _ntff_profile_via_ctypes

        hook = _ntff_profile_via_ctypes("/opt/axon/libaxon_pjrt.so")
    except Exception:
        hook = None
    mod = types.ModuleType("antenv.axon_hooks")
    mod.get_axon_ntff_profile_hook = lambda: hook
    sys.modules["antenv.axon_hooks"] = mod


def _emit(tc, io):
    nc = tc.nc
    from contextlib import ExitStack

    act_dt = FP8 if FP8_PV else BF

    ctx = ExitStack()
    with ctx:
        const = ctx.enter_context(tc.tile_pool(name="const", bufs=1))
        qk = ctx.enter_context(tc.tile_pool(name="qk", bufs=1))
        xp = ctx.enter_context(tc.tile_pool(name="xp", bufs=20))
        epool = ctx.enter_context(tc.tile_pool(name="exp", bufs=10))
        spool = ctx.enter_context(tc.tile_pool(name="small", bufs=2))
        opool = ctx.enter_context(tc.tile_pool(name="out", bufs=2))
        ps_a = ctx.enter_context(tc.tile_pool(name="psa", bufs=2, space="PSUM"))
        ps_pv = ctx.enter_context(tc.tile_pool(name="pspv", bufs=1, space="PSUM"))
        ps_b = ctx.enter_context(tc.tile_pool(name="psb", bufs=2, space="PSUM"))

        # ---------------- persistent tiles ----------------
        wk = const.tile([128, KC, F], BF, name="wk", tag="wk")
        wq = const.tile([128, KC, F], BF, name="wq", tag="wq")
        wv = const.tile([128, KC, F], BF, name="wv", tag="wv")
        wo_t = const.tile([128, 2, E], BF, name="wo", tag="wo")
        bk_t = const.tile([128, 2], F32, name="bk", tag="bk")
        bq_t = const.tile([128, 2], F32, name="bq", tag="bq")
        bv_row = const.tile([1, F], F32, name="bvrow", tag="bvrow")
        scratch = const.tile([128, 512], BF, name="scr", tag="scr")

        qcT = [qk.tile([128, T], BF, name=f"qcT{ft}", tag=f"qcT{ft}") for ft in range(2)]
        kcT = [qk.tile([128, T], BF, name=f"kcT{ft}", tag=f"kcT{ft}") for ft in range(2)]
        attnT = [qk.tile([128, T], BF, name=f"attnT{ft}", tag=f"attnT{ft}") for ft in range(2)]
        # v, augmented with a ones column per head, as 8 super-chunks of
        # 256 tk each: [tk_lo 128, plane 2, head 4, 68(64 v | 1 one | pad)]
        v2 = [const.tile([128, 2, HPC, 68], act_dt, name=f"v2_{G}", tag=f"v2_{G}")
              for G in range(8)]

        # x inputs as [128, 4, 512] half-tensor block tiles from a recycled
        # pool (20 bufs covers all live ranges); xb[(which, b, kk)] -> AP.
        # cs of length 2 loads one half of a shared 4-chunk tile (two DMAs
        # per tile for issue parallelism during the lead-in).
        xb = {}
        xhalf = {}

        def alloc_x(which, b, cs):
            key = (which, b, cs[0] // 4)
            t = xhalf.get(key)
            if t is None:
                t = xp.tile([128, 4, 512], BF, name=f"x{which}{b}", tag="x")
                xhalf[key] = t
                for i in range(4):
                    xb[(which, b, (cs[0] // 4) * 4 + i)] = t[:, i, :]
            return t

        # ---------------- DMA issue (deadline order) ----------------
        # memset scratch first so warmup matmuls read defined data
        nc.vector.memset(scratch[:], 1.0)

        def dma_x(eng, which, b, cs):
            t = alloc_x(which, b, cs)
            i0 = cs[0] % 4
            eng.dma_start(t[:, i0:i0 + len(cs), :],
                          io["x" + which][:, cs[0]:cs[0] + len(cs),
                                          b * 512:(b + 1) * 512])

        # sync + scalar are HW-DGE queues (fast); gpsimd is SW-DGE (tiny
        # loads only). scalar is used for 8 early issues only, so the ACT
        # table load still lands well before exp #0. The link runs at
        # ~358GB/s aggregate once flowing; arrival order == issue order,
        # so issues are sorted by consumer deadline.
        # only the two HW-DGE queues (sync + scalar); no gpsimd SW-DGE ring
        sy, sc = nc.sync, nc.scalar
        # wave A: everything the lead-in needs (~3MB -> done ~20us)
        sc.dma_start(wk[:, 0:4, :], io["wk"][:, 0:4, :])
        sc.dma_start(wk[:, 4:8, :], io["wk"][:, 4:8, :])
        sc.dma_start(bk_t[:], io["bk"][:, :])
        sc.dma_start(bq_t[:], io["bq"][:, :])
        dma_x(sy, "k", 0, [0, 1]); dma_x(sy, "k", 0, [2, 3])
        dma_x(sc, "k", 0, [4, 5]); dma_x(sc, "k", 0, [6, 7])
        sc.dma_start(wq[:, 0:4, :], io["wq"][:, 0:4, :])
        sc.dma_start(wq[:, 4:8, :], io["wq"][:, 4:8, :])
        dma_x(sy, "q", 0, [0, 1]); dma_x(sy, "q", 0, [2, 3])
        dma_x(sc, "q", 0, [4, 5]); dma_x(sc, "q", 0, [6, 7])
        sc.dma_start(bv_row[:], io["bv"][:, :])
        # rest of k first (kcT blocks are consumed by unit-0 score groups),
        # then v blocks (vproj under units 0-1), xq_b1 wedged in before
        # xv_b2 so unit 1's q block makes its deadline.
        dma_x(sy, "k", 1, [0, 1, 2, 3]); dma_x(sy, "k", 1, [4, 5, 6, 7])
        dma_x(sy, "k", 2, [0, 1, 2, 3]); dma_x(sy, "k", 2, [4, 5, 6, 7])
        dma_x(sy, "k", 3, [0, 1, 2, 3]); dma_x(sy, "k", 3, [4, 5, 6, 7])
        sy.dma_start(wv[:, 0:4, :], io["wv"][:, 0:4, :])
        sy.dma_start(wv[:, 4:8, :], io["wv"][:, 4:8, :])
        dma_x(sy, "v", 0, [0, 1, 2, 3]); dma_x(sy, "v", 0, [4, 5, 6, 7])
        dma_x(sy, "v", 1, [0, 1, 2, 3]); dma_x(sy, "v", 1, [4, 5, 6, 7])
        dma_x(sy, "q", 1, [0, 1, 2, 3]); dma_x(sy, "q", 1, [4, 5, 6, 7])
        dma_x(sy, "v", 2, [0, 1, 2, 3]); dma_x(sy, "v", 2, [4, 5, 6, 7])
        dma_x(sy, "v", 3, [0, 1, 2, 3]); dma_x(sy, "v", 3, [4, 5, 6, 7])
        dma_x(sy, "q", 2, [0, 1, 2, 3]); dma_x(sy, "q", 2, [4, 5, 6, 7])
        dma_x(sy, "q", 3, [0, 1, 2, 3]); dma_x(sy, "q", 3, [4, 5, 6, 7])
        sy.dma_start(wo_t[:, 0, :], io["wo"][:, 0, :])
        sy.dma_start(wo_t[:, 1, :], io["wo"][:, 1, :])

        # broadcast bv across partitions: [128, 4, 64] f32
        bv_full = const.tile([128, F], F32, name="bvbc", tag="bvbc")
        nc.gpsimd.partition_broadcast(bv_full[:], bv_row[:])
        bv_bc = bv_full[:].rearrange("p (h d) -> p h d", h=HPC)
        zt = const.tile([33, 512], F32, name="zt", tag="zt")
        nc.vector.memset(zt[:], 1.0)
        # select matrix for the tail 1/Z broadcast as one PE outer product
        # (rb = sel.T @ rc): row 0 -> partitions 0:64, row 32 -> 64:128.
        sel = const.tile([33, 128], F32, name="sel", tag="sel")
        nc.vector.memset(sel[:], 0.0)
        nc.vector.memset(sel[0:1, 0:64], 1.0)
        nc.vector.memset(sel[32:33, 64:128], 1.0)

        # ---------------- compute building blocks ----------------
        def warmup(n):
            # garbage matmuls: lift/hold the HAM clock gate during DMA
            # waits. Allocated from ps_a (unused while no unit is live /
            # rotation-safe) so they never sit inside an open ps_b group.
            for _ in range(n):
                ps = ps_a.tile([128, 1024], F32, name="warm", tag="psa")
                nc.tensor.matmul(ps[:, 0:512], scratch[:, 0:128],
                                 scratch[:, 0:512], start=True, stop=True)

        def proj_block(which, ft, b, pace=0, pace_all=False):
            # pace>0: interleave warmup matmuls between the DMA-gated
            # chunk matmuls so the PE never idles during the lead-in.
            wt, bt, dst = ((wk, bk_t, kcT) if which == "k" else (wq, bq_t, qcT))
            ps = ps_b.tile([128, 512], F32, name="psp", tag="psb")
            for kk in range(KC):
                if pace and (pace_all or kk < 6):
                    warmup(pace)
                nc.tensor.matmul(ps[:], wt[:, kk, ft * 128:(ft + 1) * 128],
                                 xb[(which, b, kk)],
                                 start=(kk == 0), stop=(kk == KC - 1))
            nc.vector.tensor_scalar_add(
                dst[ft][:, b * 512:(b + 1) * 512], ps[:], bt[:, ft:ft + 1])

        def v_tile(tt):
            G, j = tt // 2, tt % 2
            ps = ps_b.tile([128, 256], F32, name="psv", tag="psb")
            for kk in range(KC):
                nc.tensor.matmul(ps[:],
                                 xb[("v", tt // 4, kk)][:, (tt % 4) * 128:(tt % 4 + 1) * 128],
                                 wv[:, kk, :],
                                 start=(kk == 0), stop=(kk == KC - 1))
            nc.vector.tensor_tensor(
                v2[G][:, j, :, 0:Dh],
                ps[:].rearrange("p (h d) -> p h d", h=HPC),
                bv_bc[:, :, :],
                mybir.AluOpType.add)
            nc.gpsimd.memset(v2[G][:, j, :, Dh:Dh + 1], 1.0)

        units = [(0, 0), (0, 1), (0, 2), (0, 3), (1, 0), (1, 1), (1, 2), (1, 3)]
        pvs_tiles = {}
        etiles = {}

        def pv_chunk(u, G):
            p, blk = units[u]
            if u not in pvs_tiles:
                pvs_tiles[u] = [ps_pv.tile([65, 512], F32, name=f"pv{hh}", tag=f"pv{hh}")
                                for hh in range(2)]
            pvs = pvs_tiles[u]
            et = etiles[(u, G)][:].rearrange("p (j q) -> p j q", j=2)
            for hh in range(2):
                lh = p * 2 + hh
                rhs = et[:, :, hh * 512:(hh + 1) * 512]      # [128, 2, 512]
                if FP8_PV:
                    nc.tensor.matmul(pvs[hh][:], v2[G][:, :, lh, 0:Dh + 1], rhs,
                                     start=(G == 0), stop=(G == 7),
                                     perf_mode=mybir.MatmulPerfMode.DoubleRow,
                                     skip_group_check=True)
                else:
                    for j in range(2):
                        nc.tensor.matmul(pvs[hh][:], v2[G][:, j, lh, 0:Dh + 1],
                                         rhs[:, j, :],
                                         start=(G == 0 and j == 0),
                                         stop=(G == 7 and j == 1),
                                         skip_group_check=True)

        def chain(u, tail=False):
            """1/Z normalize for unit u's PV banks -> attnT (bf16).

            tail=True: latency-optimized variant for the last unit — pv
            copies on the (idle) scalar engine, the broadcast as one PE
            outer product instead of two serial gpsimd broadcasts.
            """
            p, blk = units[u]
            tq0 = blk * 512
            pvs = pvs_tiles[u]
            for hh in range(2):
                if tail and hh == 1:
                    nc.scalar.copy(zt[hh * 32:hh * 32 + 1, :], pvs[hh][64:65, :])
                else:
                    nc.vector.tensor_copy(zt[hh * 32:hh * 32 + 1, :], pvs[hh][64:65, :])
            pvcs = []
            for hh in range(2):
                pvc = spool.tile([64, 512], BF, name=f"pvc{hh}", tag=f"pvc{hh}")
                if tail:
                    nc.scalar.copy(pvc[:], pvs[hh][0:64, :])
                else:
                    nc.vector.tensor_copy(pvc[:], pvs[hh][0:64, :])
                pvcs.append(pvc)
            rc = spool.tile([33, 512], F32, name="rc", tag="rc")
            nc.vector.reciprocal_approx_fast(rc[:], zt[:])
            if tail:
                rbp = ps_b.tile([128, 512], F32, name="rbp", tag="psb")
                nc.tensor.matmul(rbp[:], sel[:], rc[:], start=True, stop=True)
                for hh in range(2):
                    nc.vector.tensor_tensor(
                        attnT[p][hh * 64:(hh + 1) * 64, tq0:tq0 + 512],
                        pvcs[hh][:], rbp[hh * 64:(hh + 1) * 64, :],
                        mybir.AluOpType.mult)
                return
            rc1 = spool.tile([1, 512], F32, name="rc1", tag="rc1")
            nc.vector.tensor_copy(rc1[:], rc[32:33, :])
            rcaps = [rc[0:1, :], rc1[:]]
            for hh in range(2):
                rb = spool.tile([64, 512], F32, name=f"rb{hh}", tag=f"rb{hh}")
                nc.gpsimd.partition_broadcast(rb[:], rcaps[hh])
                nc.vector.tensor_tensor(
                    attnT[p][hh * 64:(hh + 1) * 64, tq0:tq0 + 512],
                    pvcs[hh][:], rb[:], mybir.AluOpType.mult)

        wo_ot = {}

        def wo_half(tt, eb, tail=False):
            # one 512-wide half of an output row group: fine-grained PE
            # filler (~0.5us) so interposed work never starves the ACT.
            # tail=True: cast on the idle scalar engine and DMA each half
            # immediately so the drain isn't DVE-serialized.
            if eb == 0:
                ot = opool.tile([128, E], BF, name="ot", tag="ot")
                wo_ot[tt] = ot
            else:
                ot = wo_ot.pop(tt)
            ps = ps_b.tile([128, 512], F32, name="pswo", tag="psb")
            for fc in range(2):
                nc.tensor.matmul(ps[:], attnT[fc][:, tt * 128:(tt + 1) * 128],
                                 wo_t[:, fc, eb * 512:(eb + 1) * 512],
                                 start=(fc == 0), stop=(fc == 1))
            if tail:
                if eb == 0:
                    nc.scalar.copy(ot[:, eb * 512:(eb + 1) * 512], ps[:])
                else:
                    nc.vector.tensor_copy(ot[:, eb * 512:(eb + 1) * 512], ps[:])
                if tt == 15:
                    # very last tile: quarter DMAs on both HW queues so the
                    # final transfer isn't one 128KB single-queue drain
                    for qq, eng in ((0, nc.sync), (1, nc.scalar)):
                        c0 = eb * 512 + qq * 256
                        eng.dma_start(io["out"][tt * 128:(tt + 1) * 128,
                                                c0:c0 + 256],
                                      ot[:, c0:c0 + 256])
                else:
                    nc.sync.dma_start(
                        io["out"][tt * 128:(tt + 1) * 128, eb * 512:(eb + 1) * 512],
                        ot[:, eb * 512:(eb + 1) * 512])
                return
            nc.vector.tensor_copy(ot[:, eb * 512:(eb + 1) * 512], ps[:])
            if eb == 1:
                nc.sync.dma_start(io["out"][tt * 128:(tt + 1) * 128, :], ot[:])

        def wo_group(tt, tail=False):
            wo_half(tt, 0, tail)
            wo_half(tt, 1, tail)

        # ---------------- schedule ----------------
        # fillers placed by deadline vs DMA arrival (~358GB/s in issue
        # order); (ui, g) -> list of closures
        sched = {
            (0, 0): [lambda: proj_block("k", 0, 1)],
            (0, 2): [lambda: proj_block("k", 0, 2)],
            (0, 4): [lambda: proj_block("k", 0, 3)],
            (0, 6): [lambda: v_tile(0), lambda: v_tile(1)],
            (0, 7): [lambda: v_tile(2), lambda: v_tile(3), lambda: proj_block("q", 0, 1)],
            (1, 0): [lambda: v_tile(4), lambda: v_tile(5)],
            (1, 1): [lambda: v_tile(6), lambda: v_tile(7)],
            (1, 2): [lambda: v_tile(8), lambda: v_tile(9)],
            (1, 3): [lambda: v_tile(10), lambda: v_tile(11)],
            (1, 4): [lambda: v_tile(12), lambda: v_tile(13)],
            (1, 5): [lambda: v_tile(14), lambda: v_tile(15)],
            (1, 7): [lambda: proj_block("q", 0, 2)],
            (2, 1): [lambda: proj_block("k", 1, 0)],
            (2, 3): [lambda: proj_block("k", 1, 1)],
            (2, 4): [lambda: proj_block("q", 0, 3)],
            (2, 5): [lambda: proj_block("k", 1, 2)],
            (2, 7): [lambda: proj_block("k", 1, 3)],
            (3, 3): [lambda: proj_block("q", 1, 0)],
            (3, 5): [lambda: proj_block("q", 1, 1)],
            (4, 3): [lambda: proj_block("q", 1, 2)],
            (4, 5): [lambda: proj_block("q", 1, 3)],
            (5, 2): [lambda: wo_half(0, 0)],
            (5, 3): [lambda: wo_half(0, 1)],
            (5, 4): [lambda: wo_half(1, 0)],
            (5, 5): [lambda: wo_half(1, 1)],
            (5, 6): [lambda: wo_half(2, 0)],
            (5, 7): [lambda: wo_half(2, 1)],
            (6, 0): [lambda: wo_half(3, 0)],
            (6, 1): [lambda: wo_half(3, 1)],
            (6, 2): [lambda: wo_half(4, 0)],
            (6, 3): [lambda: wo_half(4, 1)],
            (6, 4): [lambda: wo_half(5, 0)],
            (6, 5): [lambda: wo_half(5, 1)],
            (6, 6): [lambda: wo_half(6, 0)],
            (6, 7): [lambda: wo_half(6, 1)],
            (7, 0): [lambda: wo_half(7, 0)],
            (7, 1): [lambda: wo_half(7, 1)],
            (7, 2): [lambda: wo_half(8, 0), lambda: wo_half(8, 1)],
            (7, 3): [lambda: wo_half(9, 0)],
            (7, 4): [lambda: wo_half(9, 1)],
            (7, 5): [lambda: wo_half(10, 0)],
            (7, 6): [lambda: wo_half(10, 1)],
            (7, 7): [lambda: wo_half(11, 0)],
        }

        # PV FIFO: chunk (u, G) may emit once exp (u, G+1) is emitted
        # (lag-1) and v2[G] is fully projected; chain(u) follows chunk(u,7).
        pv_q = deque()
        emitted = set()
        vcount = [0]

        def pv_ready(item, ui, g):
            kind = item[0]
            if kind == "chain":
                return True
            _, u, G = item
            if vcount[0] < 2 * (G + 1):
                return False
            need = (u, G + 1) if G < 7 else ((u + 1, 0) if u < 7 else None)
            return need is None or need in emitted

        def drain_pv(ui, g, cap=2):
            n = 0
            while pv_q and n < cap:
                item = pv_q[0]
                if not pv_ready(item, ui, g):
                    break
                pv_q.popleft()
                if item[0] == "chain":
                    chain(item[1], tail=(item[1] == 7))
                else:
                    pv_chunk(item[1], item[2])
                    n += 1

        # ---------------- emission ----------------
        warmup(8)
        proj_block("k", 0, 0, pace=2)
        proj_block("q", 0, 0, pace=2)

        orig_vtile = v_tile

        def v_tile_counted(tt):
            orig_vtile(tt)
            vcount[0] += 1

        v_tile = v_tile_counted
        # patch sched closures to use counted v_tile: rebuild lazily instead
        # (closures above captured the name `v_tile` at call time in this
        # scope, so they already see the counted version)

        for ui, (p, blk) in enumerate(units):
            for G in range(8):
                pv_q.append(("chunk", ui, G))
            pv_q.append(("chain", ui))
            for g in range(8):
                etile = epool.tile([128, 2 * 1024], act_dt, name="exp", tag="exp")
                etiles[(ui, g)] = etile
                for j2 in range(2):
                    tk = g * 2 + j2
                    ps = ps_a.tile([128, 1024], F32, name="psa", tag="psa")
                    for hh in range(2):
                        pp = hh * 64
                        nc.tensor.matmul(
                            ps[:, hh * 512:(hh + 1) * 512],
                            kcT[p][pp:pp + 64, tk * 128:(tk + 1) * 128],
                            qcT[p][pp:pp + 64, blk * 512:blk * 512 + 512],
                            start=True, stop=True)
                    nc.scalar.activation(
                        etile[:, j2 * 1024:(j2 + 1) * 1024], ps[:],
                        mybir.ActivationFunctionType.Exp, scale=1.0 / SCALE)
                emitted.add((ui, g))
                drain_pv(ui, g)
                for fn in sched.get((ui, g), ()):
                    fn()
        # tail: flush remaining PV chunks + chain; keep the PE warm with
        # dummy matmuls while the normalize chain runs, then the last Wo
        # block with DMAs split across both HW queues.
        while pv_q:
            item = pv_q.popleft()
            if item[0] == "chain":
                chain(item[1], tail=(item[1] == 7))
            else:
                pv_chunk(item[1], item[2])
        wo_half(11, 1)
        warmup(6)
        for tt in (12, 13, 14, 15):
            wo_group(tt, tail=True)


def _build():
    nc = bacc.Bacc("TRN2", target_bir_lowering=False, debug=False)
    io = {}
    for name, shape, dt in (
        ("xq", [128, KC, T], BF),
        ("xk", [128, KC, T], BF),
        ("xv", [128, KC, T], BF),
        ("wq", [128, KC, F], BF),
        ("wk", [128, KC, F], BF),
        ("wv", [128, KC, F], BF),
        ("wo", [128, 2, E], BF),
        ("bq", [128, 2], F32),
        ("bk", [128, 2], F32),
        ("bv", [1, F], F32),
    ):
        io[name] = nc.dram_tensor(name, shape, dt, kind="ExternalInput").ap()
    io["out"] = nc.dram_tensor("out", [T, E], BF, kind="ExternalOutput").ap()
    with tile.TileContext(nc) as tc:
        _emit(tc, io)
    nc.compile()
    return nc


def _fold_clr(W, b, clr):
    """q_c = q - mean_head(q) + clr  ==  x @ (C W).T + (C b + clr)."""
    W64 = W.astype(np.float64).reshape(H, Dh, E)
    W_eff = W64 - W64.mean(axis=1, keepdims=True)
    b64 = b.astype(np.float64).reshape(H, Dh)
    b_eff = b64 - b64.mean(axis=1, keepdims=True) + clr.astype(np.float64).reshape(H, Dh)
    return W_eff.reshape(E, E), b_eff.reshape(E)


_NC_CACHE = None


def _chunk3(a, nchunk):
    """[nchunk*128, M] -> [128, nchunk, M]"""
    n, m = a.shape
    return np.ascontiguousarray(
        a.reshape(nchunk, 128, m).transpose(1, 0, 2))


def kernel(**inputs):
    global _NC_CACHE
    query = np.asarray(inputs["query"], np.float32)
    key = np.asarray(inputs["key"], np.float32)
    value = np.asarray(inputs["value"], np.float32)
    mask = np.asarray(inputs["key_padding_mask"])
    Wq, bq = np.asarray(inputs["Wq"], np.float32), np.asarray(inputs["bq"], np.float32)
    Wk, bk = np.asarray(inputs["Wk"], np.float32), np.asarray(inputs["bk"], np.float32)
    Wv, bv = np.asarray(inputs["Wv"], np.float32), np.asarray(inputs["bv"], np.float32)
    Wo, bo = np.asarray(inputs["Wo"], np.float32), np.asarray(inputs["bo"], np.float32)
    cq = np.asarray(inputs["clr_bias_q"], np.float32)
    ck = np.asarray(inputs["clr_bias_k"], np.float32)
    assert not mask.any(), "kernel assumes empty key_padding_mask"

    Wq_eff, bq_eff = _fold_clr(Wq, bq, cq)
    Wk_eff, bk_eff = _fold_clr(Wk, bk, ck)

    def bf(x):
        return np.ascontiguousarray(x.astype(np.float32)).astype(BF_NP)

    in_maps = []
    for c in range(NCORES):
        b = c // 4
        fs = (c % 4) * F
        m = {
            "xq": bf(_chunk3(query[b].T, KC)),
            "xk": bf(_chunk3(key[b].T, KC)),
            "xv": bf(_chunk3(value[b].T, KC)),
            "wq": bf(_chunk3(Wq_eff[fs:fs + F].T, KC)),
            "wk": bf(_chunk3(Wk_eff[fs:fs + F].T, KC)),
            "wv": bf(_chunk3(Wv[fs:fs + F].T, KC)),
            "wo": bf(_chunk3(Wo[:, fs:fs + F].T, 2)),
            "bq": np.ascontiguousarray(
                bq_eff[fs:fs + F].reshape(2, 128).T.astype(np.float32)),
            "bk": np.ascontiguousarray(
                bk_eff[fs:fs + F].reshape(2, 128).T.astype(np.float32)),
            "bv": np.ascontiguousarray(bv[None, fs:fs + F], dtype=np.float32),
        }
        in_maps.append(m)

    if _NC_CACHE is None:
        _NC_CACHE = _build()
    nc = _NC_CACHE

    import os

    trace = bool(int(os.environ.get("KERNEL_TRACE", "0")))
    if trace:
        _install_ntff_hook()
    res = None
    last_exc = None
    out = None
    for attempt in range(4):
        try:
            res = run_bass_kernel_spmd(
                nc, in_maps, core_ids=list(range(NCORES)), trace=trace
            )
        except Exception as e:  # transient NRT_EXEC_UNIT_UNRECOVERABLE etc.
            last_exc = e
            import time

            time.sleep(2.0)
            continue
        out = np.zeros((B, T, E), np.float32)
        for c in range(NCORES):
            out[c // 4] += res.results[c]["out"].astype(np.float32)
        if np.isfinite(out).all():
            break
        out = None  # rare transient corruption: retry
    if out is None:
        if last_exc is not None and res is None:
            raise last_exc
        raise RuntimeError("kernel produced non-finite output on all attempts")
    kernel.last_results = res
    out += bo[None, None, :].astype(np.float32)
    return out


# revision 61
# speedup vs baseline: 1.0062x; 1.0062x over previous
"""Aitchison multi-head attention on 8 trn2 NeuronCores.

Per core: batch c//4, 4 heads (feature slice of 256). CLR centering is
linear -> folded into Wq/Wk + biases on host (fp64). Host sums the 4
partial output projections per batch and adds bo.

Steady state is PE-streaming-bound (~163us busy: scores 27 + PV 55 +
q/k/v proj 41 + Wo 14 + overheads, all bf16); the ACT exp train (128x
[128,1024] exps, ~1.0us each pipelined) runs just under it. Design:
- Host pre-reshapes all tensors so every load is a simple 3D AP slice;
  input DMAs are tq-block pieces issued on both HW-DGE queues
  (sync + scalar) in consumer-deadline order; the link runs ~310GB/s,
  so arrival order == issue order. First exp fires ~28us.
- A PE warmup burst (garbage matmuls into ps_a) at t=0 plus warmups
  paced between the DMA-gated lead-in projection matmuls hold the HAM
  clock gate at 2.4GHz through the lead-in (in-order PE queue: real
  work is never queued behind a stalled load).
- scores: per group the 2 heads' matmuls run concurrently via
  row-tiling (lhsT base partitions 0/64 -> tile_position auto-derive).
- PV is bf16 (fp8 measured at rel_err 2.4e-2 > the 2e-2 gate — fp8's
  relative error lands on the dominant softmax weights; a linear int8
  quant would be fine but TRN2 bass has no integer matmul). The
  softmax denominator is free: a ones-column in v makes PSUM row 64
  the Z row.
- PV chunks pop lag-1 behind their exp group from a FIFO (cap 2/slot);
  the 1/Z chain is reciprocal_approx_fast + gpsimd broadcast + DVE
  mult; the last unit uses a latency-optimized chain (pv copies on the
  idle scalar engine, broadcast as one PE sel-matrix outer product).
- k/q/v projection blocks and Wo half-groups (~0.5us filler grains)
  are spread across units by deadline; the tail keeps the PE warm with
  dummy matmuls while the final normalize chain runs, then drains the
  last Wo block with casts split across scalar+vector and per-half
  DMAs.
"""
import sys
import types

sys.path.insert(0, "/opt/trn_rl_repo")

from collections import deque

import numpy as np
import ml_dtypes

import concourse.bass as bass
import concourse.tile as tile
from concourse import bacc, mybir
from concourse.bass_utils import run_bass_kernel_spmd

B, T, E, H, Dh = 2, 2048, 1024, 16, 64
NCORES = 8
HPC = 4            # heads per core
F = HPC * Dh       # 256 features per core
SCALE = 8.0        # sqrt(Dh)
KC = E // 128      # 8 e-chunks in projections
BF = mybir.dt.bfloat16
F32 = mybir.dt.float32
FP8 = mybir.dt.float8e4
BF_NP = ml_dtypes.bfloat16

FP8_PV = False     # fp8 PV measured at rel_err 2.4e-2 (> 2e-2 gate): any
                   # fp8 in the attention path costs ~2e-2 broad noise.


def _install_ntff_hook():
    """trace=True under axon needs antenv.axon_hooks, missing in this image."""
    if "antenv.axon_hooks" in sys.modules:
        return
    try:
        from trn_agent_boot.trn_boot import _ntff_profile_via_ctypes

        hook = _ntff_profile_via_ctypes("/opt/axon/libaxon_pjrt.so")
    except Exception:
        hook = None
    mod = types.ModuleType("antenv.axon_hooks")
    mod.get_axon_ntff_profile_hook = lambda: hook
    sys.modules["antenv.axon_hooks"] = mod


def _emit(tc, io):
    nc = tc.nc
    from contextlib import ExitStack

    act_dt = FP8 if FP8_PV else BF

    ctx = ExitStack()
    with ctx:
        const = ctx.enter_context(tc.tile_pool(name="const", bufs=1))
        qk = ctx.enter_context(tc.tile_pool(name="qk", bufs=1))
        xp = ctx.enter_context(tc.tile_pool(name="xp", bufs=20))
        epool = ctx.enter_context(tc.tile_pool(name="exp", bufs=10))
        spool = ctx.enter_context(tc.tile_pool(name="small", bufs=2))
        opool = ctx.enter_context(tc.tile_pool(name="out", bufs=2))
        ps_a = ctx.enter_context(tc.tile_pool(name="psa", bufs=2, space="PSUM"))
        ps_pv = ctx.enter_context(tc.tile_pool(name="pspv", bufs=1, space="PSUM"))
        ps_b = ctx.enter_context(tc.tile_pool(name="psb", bufs=2, space="PSUM"))

        # ---------------- persistent tiles ----------------
        wk = const.tile([128, KC, F], BF, name="wk", tag="wk")
        wq = const.tile([128, KC, F], BF, name="wq", tag="wq")
        wv = const.tile([128, KC, F], BF, name="wv", tag="wv")
        wo_t = const.tile([128, 2, E], BF, name="wo", tag="wo")
        bk_t = const.tile([128, 2], F32, name="bk", tag="bk")
        bq_t = const.tile([128, 2], F32, name="bq", tag="bq")
        bv_row = const.tile([1, F], F32, name="bvrow", tag="bvrow")
        scratch = const.tile([128, 512], BF, name="scr", tag="scr")

        qcT = [qk.tile([128, T], BF, name=f"qcT{ft}", tag=f"qcT{ft}") for ft in range(2)]
        kcT = [qk.tile([128, T], BF, name=f"kcT{ft}", tag=f"kcT{ft}") for ft in range(2)]
        attnT = [qk.tile([128, T], BF, name=f"attnT{ft}", tag=f"attnT{ft}") for ft in range(2)]
        # v, augmented with a ones column per head, as 8 super-chunks of
        # 256 tk each: [tk_lo 128, plane 2, head 4, 68(64 v | 1 one | pad)]
        v2 = [const.tile([128, 2, HPC, 68], act_dt, name=f"v2_{G}", tag=f"v2_{G}")
              for G in range(8)]

        # x inputs as [128, 4, 512] half-tensor block tiles from a recycled
        # pool (20 bufs covers all live ranges); xb[(which, b, kk)] -> AP.
        # cs of length 2 loads one half of a shared 4-chunk tile (two DMAs
        # per tile for issue parallelism during the lead-in).
        xb = {}
        xhalf = {}

        def alloc_x(which, b, cs):
            key = (which, b, cs[0] // 4)
            t = xhalf.get(key)
            if t is None:
                t = xp.tile([128, 4, 512], BF, name=f"x{which}{b}", tag="x")
                xhalf[key] = t
                for i in range(4):
                    xb[(which, b, (cs[0] // 4) * 4 + i)] = t[:, i, :]
            return t

        # ---------------- DMA issue (deadline order) ----------------
        # memset scratch first so warmup matmuls read defined data
        nc.vector.memset(scratch[:], 1.0)

        def dma_x(eng, which, b, cs):
            t = alloc_x(which, b, cs)
            i0 = cs[0] % 4
            eng.dma_start(t[:, i0:i0 + len(cs), :],
                          io["x" + which][:, cs[0]:cs[0] + len(cs),
                                          b * 512:(b + 1) * 512])

        # sync + scalar are HW-DGE queues (fast); gpsimd is SW-DGE (tiny
        # loads only). scalar is used for 8 early issues only, so the ACT
        # table load still lands well before exp #0. The link runs at
        # ~358GB/s aggregate once flowing; arrival order == issue order,
        # so issues are sorted by consumer deadline.
        # only the two HW-DGE queues (sync + scalar); no gpsimd SW-DGE ring
        sy, sc = nc.sync, nc.scalar
        # wave A: everything the lead-in needs (~3MB -> done ~20us)
        sc.dma_start(wk[:, 0:4, :], io["wk"][:, 0:4, :])
        sc.dma_start(wk[:, 4:8, :], io["wk"][:, 4:8, :])
        sc.dma_start(bk_t[:], io["bk"][:, :])
        sc.dma_start(bq_t[:], io["bq"][:, :])
        dma_x(sy, "k", 0, [0, 1]); dma_x(sy, "k", 0, [2, 3])
        dma_x(sc, "k", 0, [4, 5]); dma_x(sc, "k", 0, [6, 7])
        sc.dma_start(wq[:, 0:4, :], io["wq"][:, 0:4, :])
        sc.dma_start(wq[:, 4:8, :], io["wq"][:, 4:8, :])
        dma_x(sy, "q", 0, [0, 1]); dma_x(sy, "q", 0, [2, 3])
        dma_x(sc, "q", 0, [4, 5]); dma_x(sc, "q", 0, [6, 7])
        sc.dma_start(bv_row[:], io["bv"][:, :])
        # rest of k first (kcT blocks are consumed by unit-0 score groups),
        # then v blocks (vproj under units 0-1), xq_b1 wedged in before
        # xv_b2 so unit 1's q block makes its deadline.
        dma_x(sy, "k", 1, [0, 1, 2, 3]); dma_x(sy, "k", 1, [4, 5, 6, 7])
        dma_x(sy, "k", 2, [0, 1, 2, 3]); dma_x(sy, "k", 2, [4, 5, 6, 7])
        dma_x(sy, "k", 3, [0, 1, 2, 3]); dma_x(sy, "k", 3, [4, 5, 6, 7])
        sy.dma_start(wv[:, 0:4, :], io["wv"][:, 0:4, :])
        sy.dma_start(wv[:, 4:8, :], io["wv"][:, 4:8, :])
        dma_x(sy, "v", 0, [0, 1, 2, 3]); dma_x(sy, "v", 0, [4, 5, 6, 7])
        dma_x(sy, "v", 1, [0, 1, 2, 3]); dma_x(sy, "v", 1, [4, 5, 6, 7])
        dma_x(sy, "q", 1, [0, 1, 2, 3]); dma_x(sy, "q", 1, [4, 5, 6, 7])
        dma_x(sy, "v", 2, [0, 1, 2, 3]); dma_x(sy, "v", 2, [4, 5, 6, 7])
        dma_x(sy, "v", 3, [0, 1, 2, 3]); dma_x(sy, "v", 3, [4, 5, 6, 7])
        dma_x(sy, "q", 2, [0, 1, 2, 3]); dma_x(sy, "q", 2, [4, 5, 6, 7])
        dma_x(sy, "q", 3, [0, 1, 2, 3]); dma_x(sy, "q", 3, [4, 5, 6, 7])
        sy.dma_start(wo_t[:, 0, :], io["wo"][:, 0, :])
        sy.dma_start(wo_t[:, 1, :], io["wo"][:, 1, :])

        # broadcast bv across partitions: [128, 4, 64] f32
        bv_full = const.tile([128, F], F32, name="bvbc", tag="bvbc")
        nc.gpsimd.partition_broadcast(bv_full[:], bv_row[:])
        bv_bc = bv_full[:].rearrange("p (h d) -> p h d", h=HPC)
        zt = const.tile([33, 512], F32, name="zt", tag="zt")
        nc.vector.memset(zt[:], 1.0)
        # select matrix for the tail 1/Z broadcast as one PE outer product
        # (rb = sel.T @ rc): row 0 -> partitions 0:64, row 32 -> 64:128.
        sel = const.tile([33, 128], F32, name="sel", tag="sel")
        nc.vector.memset(sel[:], 0.0)
        nc.vector.memset(sel[0:1, 0:64], 1.0)
        nc.vector.memset(sel[32:33, 64:128], 1.0)

        # ---------------- compute building blocks ----------------
        def warmup(n):
            # garbage matmuls: lift/hold the HAM clock gate during DMA
            # waits. Allocated from ps_a (unused while no unit is live /
            # rotation-safe) so they never sit inside an open ps_b group.
            for _ in range(n):
                ps = ps_a.tile([128, 1024], F32, name="warm", tag="psa")
                nc.tensor.matmul(ps[:, 0:512], scratch[:, 0:128],
                                 scratch[:, 0:512], start=True, stop=True)

        def proj_block(which, ft, b, pace=0, pace_all=False):
            # pace>0: interleave warmup matmuls between the DMA-gated
            # chunk matmuls so the PE never idles during the lead-in.
            wt, bt, dst = ((wk, bk_t, kcT) if which == "k" else (wq, bq_t, qcT))
            ps = ps_b.tile([128, 512], F32, name="psp", tag="psb")
            for kk in range(KC):
                if pace and (pace_all or kk < 6):
                    warmup(pace)
                nc.tensor.matmul(ps[:], wt[:, kk, ft * 128:(ft + 1) * 128],
                                 xb[(which, b, kk)],
                                 start=(kk == 0), stop=(kk == KC - 1))
            nc.vector.tensor_scalar_add(
                dst[ft][:, b * 512:(b + 1) * 512], ps[:], bt[:, ft:ft + 1])

        def v_tile(tt):
            G, j = tt // 2, tt % 2
            ps = ps_b.tile([128, 256], F32, name="psv", tag="psb")
            for kk in range(KC):
                nc.tensor.matmul(ps[:],
                                 xb[("v", tt // 4, kk)][:, (tt % 4) * 128:(tt % 4 + 1) * 128],
                                 wv[:, kk, :],
                                 start=(kk == 0), stop=(kk == KC - 1))
            nc.vector.tensor_tensor(
                v2[G][:, j, :, 0:Dh],
                ps[:].rearrange("p (h d) -> p h d", h=HPC),
                bv_bc[:, :, :],
                mybir.AluOpType.add)
            nc.gpsimd.memset(v2[G][:, j, :, Dh:Dh + 1], 1.0)

        units = [(0, 0), (0, 1), (0, 2), (0, 3), (1, 0), (1, 1), (1, 2), (1, 3)]
        pvs_tiles = {}
        etiles = {}

        def pv_chunk(u, G):
            p, blk = units[u]
            if u not in pvs_tiles:
                pvs_tiles[u] = [ps_pv.tile([65, 512], F32, name=f"pv{hh}", tag=f"pv{hh}")
                                for hh in range(2)]
            pvs = pvs_tiles[u]
            et = etiles[(u, G)][:].rearrange("p (j q) -> p j q", j=2)
            for hh in range(2):
                lh = p * 2 + hh
                rhs = et[:, :, hh * 512:(hh + 1) * 512]      # [128, 2, 512]
                if FP8_PV:
                    nc.tensor.matmul(pvs[hh][:], v2[G][:, :, lh, 0:Dh + 1], rhs,
                                     start=(G == 0), stop=(G == 7),
                                     perf_mode=mybir.MatmulPerfMode.DoubleRow,
                                     skip_group_check=True)
                else:
                    for j in range(2):
                        nc.tensor.matmul(pvs[hh][:], v2[G][:, j, lh, 0:Dh + 1],
                                         rhs[:, j, :],
                                         start=(G == 0 and j == 0),
                                         stop=(G == 7 and j == 1),
                                         skip_group_check=True)

        def chain(u, tail=False):
            """1/Z normalize for unit u's PV banks -> attnT (bf16).

            tail=True: latency-optimized variant for the last unit — pv
            copies on the (idle) scalar engine, the broadcast as one PE
            outer product instead of two serial gpsimd broadcasts.
            """
            p, blk = units[u]
            tq0 = blk * 512
            pvs = pvs_tiles[u]
            for hh in range(2):
                if tail and hh == 1:
                    nc.scalar.copy(zt[hh * 32:hh * 32 + 1, :], pvs[hh][64:65, :])
                else:
                    nc.vector.tensor_copy(zt[hh * 32:hh * 32 + 1, :], pvs[hh][64:65, :])
            pvcs = []
            for hh in range(2):
                pvc = spool.tile([64, 512], BF, name=f"pvc{hh}", tag=f"pvc{hh}")
                if tail:
                    nc.scalar.copy(pvc[:], pvs[hh][0:64, :])
                else:
                    nc.vector.tensor_copy(pvc[:], pvs[hh][0:64, :])
                pvcs.append(pvc)
            rc = spool.tile([33, 512], F32, name="rc", tag="rc")
            nc.vector.reciprocal_approx_fast(rc[:], zt[:])
            if tail:
                rbp = ps_b.tile([128, 512], F32, name="rbp", tag="psb")
                nc.tensor.matmul(rbp[:], sel[:], rc[:], start=True, stop=True)
                for hh in range(2):
                    nc.vector.tensor_tensor(
                        attnT[p][hh * 64:(hh + 1) * 64, tq0:tq0 + 512],
                        pvcs[hh][:], rbp[hh * 64:(hh + 1) * 64, :],
                        mybir.AluOpType.mult)
                return
            rc1 = spool.tile([1, 512], F32, name="rc1", tag="rc1")
            nc.vector.tensor_copy(rc1[:], rc[32:33, :])
            rcaps = [rc[0:1, :], rc1[:]]
            for hh in range(2):
                rb = spool.tile([64, 512], F32, name=f"rb{hh}", tag=f"rb{hh}")
                nc.gpsimd.partition_broadcast(rb[:], rcaps[hh])
                nc.vector.tensor_tensor(
                    attnT[p][hh * 64:(hh + 1) * 64, tq0:tq0 + 512],
                    pvcs[hh][:], rb[:], mybir.AluOpType.mult)

        wo_ot = {}

        def wo_half(tt, eb, tail=False):
            # one 512-wide half of an output row group: fine-grained PE
            # filler (~0.5us) so interposed work never starves the ACT.
            # tail=True: cast on the idle scalar engine and DMA each half
            # immediately so the drain isn't DVE-serialized.
            if eb == 0:
                ot = opool.tile([128, E], BF, name="ot", tag="ot")
                wo_ot[tt] = ot
            else:
                ot = wo_ot.pop(tt)
            ps = ps_b.tile([128, 512], F32, name="pswo", tag="psb")
            for fc in range(2):
                nc.tensor.matmul(ps[:], attnT[fc][:, tt * 128:(tt + 1) * 128],
                                 wo_t[:, fc, eb * 512:(eb + 1) * 512],
                                 start=(fc == 0), stop=(fc == 1))
            if tail:
                if eb == 0:
                    nc.scalar.copy(ot[:, eb * 512:(eb + 1) * 512], ps[:])
                else:
                    nc.vector.tensor_copy(ot[:, eb * 512:(eb + 1) * 512], ps[:])
                nc.sync.dma_start(
                    io["out"][tt * 128:(tt + 1) * 128, eb * 512:(eb + 1) * 512],
                    ot[:, eb * 512:(eb + 1) * 512])
                return
            nc.vector.tensor_copy(ot[:, eb * 512:(eb + 1) * 512], ps[:])
            if eb == 1:
                nc.sync.dma_start(io["out"][tt * 128:(tt + 1) * 128, :], ot[:])

        def wo_group(tt, tail=False):
            wo_half(tt, 0, tail)
            wo_half(tt, 1, tail)

        # ---------------- schedule ----------------
        # fillers placed by deadline vs DMA arrival (~358GB/s in issue
        # order); (ui, g) -> list of closures
        sched = {
            (0, 0): [lambda: proj_block("k", 0, 1)],
            (0, 2): [lambda: proj_block("k", 0, 2)],
            (0, 4): [lambda: proj_block("k", 0, 3)],
            (0, 6): [lambda: v_tile(0), lambda: v_tile(1)],
            (0, 7): [lambda: v_tile(2), lambda: v_tile(3), lambda: proj_block("q", 0, 1)],
            (1, 0): [lambda: v_tile(4), lambda: v_tile(5)],
            (1, 1): [lambda: v_tile(6), lambda: v_tile(7)],
            (1, 2): [lambda: v_tile(8), lambda: v_tile(9)],
            (1, 3): [lambda: v_tile(10), lambda: v_tile(11)],
            (1, 4): [lambda: v_tile(12), lambda: v_tile(13)],
            (1, 5): [lambda: v_tile(14), lambda: v_tile(15)],
            (1, 7): [lambda: proj_block("q", 0, 2)],
            (2, 1): [lambda: proj_block("k", 1, 0)],
            (2, 3): [lambda: proj_block("k", 1, 1)],
            (2, 4): [lambda: proj_block("q", 0, 3)],
            (2, 5): [lambda: proj_block("k", 1, 2)],
            (2, 7): [lambda: proj_block("k", 1, 3)],
            (3, 3): [lambda: proj_block("q", 1, 0)],
            (3, 5): [lambda: proj_block("q", 1, 1)],
            (4, 3): [lambda: proj_block("q", 1, 2)],
            (4, 5): [lambda: proj_block("q", 1, 3)],
            (5, 2): [lambda: wo_half(0, 0)],
            (5, 3): [lambda: wo_half(0, 1)],
            (5, 4): [lambda: wo_half(1, 0)],
            (5, 5): [lambda: wo_half(1, 1)],
            (5, 6): [lambda: wo_half(2, 0)],
            (5, 7): [lambda: wo_half(2, 1)],
            (6, 0): [lambda: wo_half(3, 0)],
            (6, 1): [lambda: wo_half(3, 1)],
            (6, 2): [lambda: wo_half(4, 0)],
            (6, 3): [lambda: wo_half(4, 1)],
            (6, 4): [lambda: wo_half(5, 0)],
            (6, 5): [lambda: wo_half(5, 1)],
            (6, 6): [lambda: wo_half(6, 0)],
            (6, 7): [lambda: wo_half(6, 1)],
            (7, 0): [lambda: wo_half(7, 0)],
            (7, 1): [lambda: wo_half(7, 1)],
            (7, 2): [lambda: wo_half(8, 0), lambda: wo_half(8, 1)],
            (7, 3): [lambda: wo_half(9, 0)],
            (7, 4): [lambda: wo_half(9, 1)],
            (7, 5): [lambda: wo_half(10, 0)],
            (7, 6): [lambda: wo_half(10, 1)],
            (7, 7): [lambda: wo_half(11, 0, True)],
        }

        # PV FIFO: chunk (u, G) may emit once exp (u, G+1) is emitted
        # (lag-1) and v2[G] is fully projected; chain(u) follows chunk(u,7).
        pv_q = deque()
        emitted = set()
        vcount = [0]

        def pv_ready(item, ui, g):
            kind = item[0]
            if kind == "chain":
                return True
            _, u, G = item
            if vcount[0] < 2 * (G + 1):
                return False
            need = (u, G + 1) if G < 7 else ((u + 1, 0) if u < 7 else None)
            return need is None or need in emitted

        def drain_pv(ui, g, cap=2):
            n = 0
            while pv_q and n < cap:
                item = pv_q[0]
                if not pv_ready(item, ui, g):
                    break
                pv_q.popleft()
                if item[0] == "chain":
                    chain(item[1], tail=(item[1] == 7))
                else:
                    pv_chunk(item[1], item[2])
                    n += 1

        # ---------------- emission ----------------
        warmup(8)
        proj_block("k", 0, 0, pace=2)
        proj_block("q", 0, 0, pace=2)

        orig_vtile = v_tile

        def v_tile_counted(tt):
            orig_vtile(tt)
            vcount[0] += 1

        v_tile = v_tile_counted
        # patch sched closures to use counted v_tile: rebuild lazily instead
        # (closures above captured the name `v_tile` at call time in this
        # scope, so they already see the counted version)

        for ui, (p, blk) in enumerate(units):
            for G in range(8):
                pv_q.append(("chunk", ui, G))
            pv_q.append(("chain", ui))
            for g in range(8):
                etile = epool.tile([128, 2 * 1024], act_dt, name="exp", tag="exp")
                etiles[(ui, g)] = etile
                for j2 in range(2):
                    tk = g * 2 + j2
                    ps = ps_a.tile([128, 1024], F32, name="psa", tag="psa")
                    for hh in range(2):
                        pp = hh * 64
                        nc.tensor.matmul(
                            ps[:, hh * 512:(hh + 1) * 512],
                            kcT[p][pp:pp + 64, tk * 128:(tk + 1) * 128],
                            qcT[p][pp:pp + 64, blk * 512:blk * 512 + 512],
                            start=True, stop=True)
                    nc.scalar.activation(
                        etile[:, j2 * 1024:(j2 + 1) * 1024], ps[:],
                        mybir.ActivationFunctionType.Exp, scale=1.0 / SCALE)
                emitted.add((ui, g))
                drain_pv(ui, g)
                for fn in sched.get((ui, g), ()):
                    fn()
        # tail: flush remaining PV chunks + chain; keep the PE warm with
        # dummy matmuls while the normalize chain runs, then the last Wo
        # block with DMAs split across both HW queues.
        while pv_q:
            item = pv_q.popleft()
            if item[0] == "chain":
                chain(item[1], tail=(item[1] == 7))
            else:
                pv_chunk(item[1], item[2])
        wo_half(11, 1, True)
        warmup(12)
        for tt in (12, 13, 14, 15):
            wo_group(tt, tail=True)


def _build():
    nc = bacc.Bacc("TRN2", target_bir_lowering=False, debug=False)
    io = {}
    for name, shape, dt in (
        ("xq", [128, KC, T], BF),
        ("xk", [128, KC, T], BF),
        ("xv", [128, KC, T], BF),
        ("wq", [128, KC, F], BF),
        ("wk", [128, KC, F], BF),
        ("wv", [128, KC, F], BF),
        ("wo", [128, 2, E], BF),
        ("bq", [128, 2], F32),
        ("bk", [128, 2], F32),
        ("bv", [1, F], F32),
    ):
        io[name] = nc.dram_tensor(name, shape, dt, kind="ExternalInput").ap()
    io["out"] = nc.dram_tensor("out", [T, E], BF, kind="ExternalOutput").ap()
    with tile.TileContext(nc) as tc:
        _emit(tc, io)
    nc.compile()
    return nc


def _fold_clr(W, b, clr):
    """q_c = q - mean_head(q) + clr  ==  x @ (C W).T + (C b + clr)."""
    W64 = W.astype(np.float64).reshape(H, Dh, E)
    W_eff = W64 - W64.mean(axis=1, keepdims=True)
    b64 = b.astype(np.float64).reshape(H, Dh)
    b_eff = b64 - b64.mean(axis=1, keepdims=True) + clr.astype(np.float64).reshape(H, Dh)
    return W_eff.reshape(E, E), b_eff.reshape(E)


_NC_CACHE = None


def _chunk3(a, nchunk):
    """[nchunk*128, M] -> [128, nchunk, M]"""
    n, m = a.shape
    return np.ascontiguousarray(
        a.reshape(nchunk, 128, m).transpose(1, 0, 2))


def kernel(**inputs):
    global _NC_CACHE
    query = np.asarray(inputs["query"], np.float32)
    key = np.asarray(inputs["key"], np.float32)
    value = np.asarray(inputs["value"], np.float32)
    mask = np.asarray(inputs["key_padding_mask"])
    Wq, bq = np.asarray(inputs["Wq"], np.float32), np.asarray(inputs["bq"], np.float32)
    Wk, bk = np.asarray(inputs["Wk"], np.float32), np.asarray(inputs["bk"], np.float32)
    Wv, bv = np.asarray(inputs["Wv"], np.float32), np.asarray(inputs["bv"], np.float32)
    Wo, bo = np.asarray(inputs["Wo"], np.float32), np.asarray(inputs["bo"], np.float32)
    cq = np.asarray(inputs["clr_bias_q"], np.float32)
    ck = np.asarray(inputs["clr_bias_k"], np.float32)
    assert not mask.any(), "kernel assumes empty key_padding_mask"

    Wq_eff, bq_eff = _fold_clr(Wq, bq, cq)
    Wk_eff, bk_eff = _fold_clr(Wk, bk, ck)

    def bf(x):
        return np.ascontiguousarray(x.astype(np.float32)).astype(BF_NP)

    in_maps = []
    for c in range(NCORES):
        b = c // 4
        fs = (c % 4) * F
        m = {
            "xq": bf(_chunk3(query[b].T, KC)),
            "xk": bf(_chunk3(key[b].T, KC)),
            "xv": bf(_chunk3(value[b].T, KC)),
            "wq": bf(_chunk3(Wq_eff[fs:fs + F].T, KC)),
            "wk": bf(_chunk3(Wk_eff[fs:fs + F].T, KC)),
            "wv": bf(_chunk3(Wv[fs:fs + F].T, KC)),
            "wo": bf(_chunk3(Wo[:, fs:fs + F].T, 2)),
            "bq": np.ascontiguousarray(
                bq_eff[fs:fs + F].reshape(2, 128).T.astype(np.float32)),
            "bk": np.ascontiguousarray(
                bk_eff[fs:fs + F].reshape(2, 128).T.astype(np.float32)),
            "bv": np.ascontiguousarray(bv[None, fs:fs + F], dtype=np.float32),
        }
        in_maps.append(m)

    if _NC_CACHE is None:
        _NC_CACHE = _build()
    nc = _NC_CACHE

    import os

    trace = bool(int(os.environ.get("KERNEL_TRACE", "0")))
    if trace:
        _install_ntff_hook()
    res = None
    last_exc = None
    out = None
    for attempt in range(4):
        try:
            res = run_bass_kernel_spmd(
                nc, in_maps, core_ids=list(range(NCORES)), trace=trace
            )
        except Exception as e:  # transient NRT_EXEC_UNIT_UNRECOVERABLE etc.
            last_exc = e
            import time

            time.sleep(2.0)
            continue
        out = np.zeros((B, T, E), np.float32)
        for c in range(NCORES):
            out[c // 4] += res.results[c]["out"].astype(np.float32)
        if np.isfinite(out).all():
            break
        out = None  # rare transient corruption: retry
    if out is None:
        if last_exc is not None and res is None:
            raise last_exc
        raise RuntimeError("kernel produced non-finite output on all attempts")
    kernel.last_results = res
    out += bo[None, None, :].astype(np.float32)
    return out


# revision 62
# speedup vs baseline: 1.0179x; 1.0116x over previous
"""Aitchison multi-head attention on 8 trn2 NeuronCores.

Per core: batch c//4, 4 heads (feature slice of 256). CLR centering is
linear -> folded into Wq/Wk + biases on host (fp64). Host sums the 4
partial output projections per batch and adds bo.

Steady state is PE-streaming-bound (~163us busy: scores 27 + PV 55 +
q/k/v proj 41 + Wo 14 + overheads, all bf16); the ACT exp train (128x
[128,1024] exps, ~1.0us each pipelined) runs just under it. Design:
- Host pre-reshapes all tensors so every load is a simple 3D AP slice;
  input DMAs are tq-block pieces issued on both HW-DGE queues
  (sync + scalar) in consumer-deadline order; the link runs ~310GB/s,
  so arrival order == issue order. First exp fires ~28us.
- A PE warmup burst (garbage matmuls into ps_a) at t=0 plus warmups
  paced between the DMA-gated lead-in projection matmuls hold the HAM
  clock gate at 2.4GHz through the lead-in (in-order PE queue: real
  work is never queued behind a stalled load).
- scores: per group the 2 heads' matmuls run concurrently via
  row-tiling (lhsT base partitions 0/64 -> tile_position auto-derive).
- PV is bf16 (fp8 measured at rel_err 2.4e-2 > the 2e-2 gate — fp8's
  relative error lands on the dominant softmax weights; a linear int8
  quant would be fine but TRN2 bass has no integer matmul). The
  softmax denominator is free: a ones-column in v makes PSUM row 64
  the Z row.
- PV chunks pop lag-1 behind their exp group from a FIFO (cap 2/slot);
  the 1/Z chain is reciprocal_approx_fast + gpsimd broadcast + DVE
  mult; the last unit uses a latency-optimized chain (pv copies on the
  idle scalar engine, broadcast as one PE sel-matrix outer product).
- k/q/v projection blocks and Wo half-groups (~0.5us filler grains)
  are spread across units by deadline; the tail keeps the PE warm with
  dummy matmuls while the final normalize chain runs, then drains the
  last Wo block with casts split across scalar+vector and per-half
  DMAs.
"""
import sys
import types

sys.path.insert(0, "/opt/trn_rl_repo")

from collections import deque

import numpy as np
import ml_dtypes

import concourse.bass as bass
import concourse.tile as tile
from concourse import bacc, mybir
from concourse.bass_utils import run_bass_kernel_spmd

B, T, E, H, Dh = 2, 2048, 1024, 16, 64
NCORES = 8
HPC = 4            # heads per core
F = HPC * Dh       # 256 features per core
SCALE = 8.0        # sqrt(Dh)
KC = E // 128      # 8 e-chunks in projections
BF = mybir.dt.bfloat16
F32 = mybir.dt.float32
FP8 = mybir.dt.float8e4
BF_NP = ml_dtypes.bfloat16

FP8_PV = False     # fp8 PV measured at rel_err 2.4e-2 (> 2e-2 gate): any
                   # fp8 in the attention path costs ~2e-2 broad noise.


def _install_ntff_hook():
    """trace=True under axon needs antenv.axon_hooks, missing in this image."""
    if "antenv.axon_hooks" in sys.modules:
        return
    try:
        from trn_agent_boot.trn_boot import _ntff_profile_via_ctypes

        hook = _ntff_profile_via_ctypes("/opt/axon/libaxon_pjrt.so")
    except Exception:
        hook = None
    mod = types.ModuleType("antenv.axon_hooks")
    mod.get_axon_ntff_profile_hook = lambda: hook
    sys.modules["antenv.axon_hooks"] = mod


def _emit(tc, io):
    nc = tc.nc
    from contextlib import ExitStack

    act_dt = FP8 if FP8_PV else BF

    ctx = ExitStack()
    with ctx:
        const = ctx.enter_context(tc.tile_pool(name="const", bufs=1))
        qk = ctx.enter_context(tc.tile_pool(name="qk", bufs=1))
        xp = ctx.enter_context(tc.tile_pool(name="xp", bufs=20))
        epool = ctx.enter_context(tc.tile_pool(name="exp", bufs=10))
        spool = ctx.enter_context(tc.tile_pool(name="small", bufs=2))
        opool = ctx.enter_context(tc.tile_pool(name="out", bufs=2))
        ps_a = ctx.enter_context(tc.tile_pool(name="psa", bufs=2, space="PSUM"))
        ps_pv = ctx.enter_context(tc.tile_pool(name="pspv", bufs=1, space="PSUM"))
        ps_b = ctx.enter_context(tc.tile_pool(name="psb", bufs=2, space="PSUM"))

        # ---------------- persistent tiles ----------------
        wk = const.tile([128, KC, F], BF, name="wk", tag="wk")
        wq = const.tile([128, KC, F], BF, name="wq", tag="wq")
        wv = const.tile([128, KC, F], BF, name="wv", tag="wv")
        wo_t = const.tile([128, 2, E], BF, name="wo", tag="wo")
        bk_t = const.tile([128, 2], F32, name="bk", tag="bk")
        bq_t = const.tile([128, 2], F32, name="bq", tag="bq")
        bv_row = const.tile([1, F], F32, name="bvrow", tag="bvrow")
        scratch = const.tile([128, 512], BF, name="scr", tag="scr")

        qcT = [qk.tile([128, T], BF, name=f"qcT{ft}", tag=f"qcT{ft}") for ft in range(2)]
        kcT = [qk.tile([128, T], BF, name=f"kcT{ft}", tag=f"kcT{ft}") for ft in range(2)]
        attnT = [qk.tile([128, T], BF, name=f"attnT{ft}", tag=f"attnT{ft}") for ft in range(2)]
        # v, augmented with a ones column per head, as 8 super-chunks of
        # 256 tk each: [tk_lo 128, plane 2, head 4, 68(64 v | 1 one | pad)]
        v2 = [const.tile([128, 2, HPC, 68], act_dt, name=f"v2_{G}", tag=f"v2_{G}")
              for G in range(8)]

        # x inputs as [128, 4, 512] half-tensor block tiles from a recycled
        # pool (20 bufs covers all live ranges); xb[(which, b, kk)] -> AP.
        # cs of length 2 loads one half of a shared 4-chunk tile (two DMAs
        # per tile for issue parallelism during the lead-in).
        xb = {}
        xhalf = {}

        def alloc_x(which, b, cs):
            key = (which, b, cs[0] // 4)
            t = xhalf.get(key)
            if t is None:
                t = xp.tile([128, 4, 512], BF, name=f"x{which}{b}", tag="x")
                xhalf[key] = t
                for i in range(4):
                    xb[(which, b, (cs[0] // 4) * 4 + i)] = t[:, i, :]
            return t

        # ---------------- DMA issue (deadline order) ----------------
        # memset scratch first so warmup matmuls read defined data
        nc.vector.memset(scratch[:], 1.0)

        def dma_x(eng, which, b, cs):
            t = alloc_x(which, b, cs)
            i0 = cs[0] % 4
            eng.dma_start(t[:, i0:i0 + len(cs), :],
                          io["x" + which][:, cs[0]:cs[0] + len(cs),
                                          b * 512:(b + 1) * 512])

        # sync + scalar are HW-DGE queues (fast); gpsimd is SW-DGE (tiny
        # loads only). scalar is used for 8 early issues only, so the ACT
        # table load still lands well before exp #0. The link runs at
        # ~358GB/s aggregate once flowing; arrival order == issue order,
        # so issues are sorted by consumer deadline.
        # only the two HW-DGE queues (sync + scalar); no gpsimd SW-DGE ring
        sy, sc = nc.sync, nc.scalar
        # wave A: everything the lead-in needs (~3MB). Per-queue rings
        # drain in order and share the link ~fairly, so wave-A bytes are
        # BALANCED across the two queues (~1.5MB each) — the last xq_b0
        # chunk gates exp #0.
        sc.dma_start(wk[:, 0:4, :], io["wk"][:, 0:4, :])
        sc.dma_start(wk[:, 4:8, :], io["wk"][:, 4:8, :])
        sc.dma_start(bk_t[:], io["bk"][:, :])
        sc.dma_start(bq_t[:], io["bq"][:, :])
        dma_x(sy, "k", 0, [0, 1]); dma_x(sy, "k", 0, [2, 3])
        dma_x(sc, "k", 0, [4, 5]); dma_x(sc, "k", 0, [6, 7])
        sy.dma_start(wq[:, 0:4, :], io["wq"][:, 0:4, :])
        sy.dma_start(wq[:, 4:8, :], io["wq"][:, 4:8, :])
        dma_x(sy, "q", 0, [0, 1]); dma_x(sy, "q", 0, [2, 3])
        dma_x(sc, "q", 0, [4, 5]); dma_x(sc, "q", 0, [6, 7])
        sc.dma_start(bv_row[:], io["bv"][:, :])
        # rest of k first (kcT blocks are consumed by unit-0 score groups),
        # then v blocks (vproj under units 0-1), xq_b1 wedged in before
        # xv_b2 so unit 1's q block makes its deadline.
        dma_x(sy, "k", 1, [0, 1, 2, 3]); dma_x(sy, "k", 1, [4, 5, 6, 7])
        dma_x(sy, "k", 2, [0, 1, 2, 3]); dma_x(sy, "k", 2, [4, 5, 6, 7])
        dma_x(sy, "k", 3, [0, 1, 2, 3]); dma_x(sy, "k", 3, [4, 5, 6, 7])
        sy.dma_start(wv[:, 0:4, :], io["wv"][:, 0:4, :])
        sy.dma_start(wv[:, 4:8, :], io["wv"][:, 4:8, :])
        dma_x(sy, "v", 0, [0, 1, 2, 3]); dma_x(sy, "v", 0, [4, 5, 6, 7])
        dma_x(sy, "v", 1, [0, 1, 2, 3]); dma_x(sy, "v", 1, [4, 5, 6, 7])
        dma_x(sy, "q", 1, [0, 1, 2, 3]); dma_x(sy, "q", 1, [4, 5, 6, 7])
        dma_x(sy, "v", 2, [0, 1, 2, 3]); dma_x(sy, "v", 2, [4, 5, 6, 7])
        dma_x(sy, "v", 3, [0, 1, 2, 3]); dma_x(sy, "v", 3, [4, 5, 6, 7])
        dma_x(sy, "q", 2, [0, 1, 2, 3]); dma_x(sy, "q", 2, [4, 5, 6, 7])
        dma_x(sy, "q", 3, [0, 1, 2, 3]); dma_x(sy, "q", 3, [4, 5, 6, 7])
        sy.dma_start(wo_t[:, 0, :], io["wo"][:, 0, :])
        sy.dma_start(wo_t[:, 1, :], io["wo"][:, 1, :])

        # broadcast bv across partitions: [128, 4, 64] f32
        bv_full = const.tile([128, F], F32, name="bvbc", tag="bvbc")
        nc.gpsimd.partition_broadcast(bv_full[:], bv_row[:])
        bv_bc = bv_full[:].rearrange("p (h d) -> p h d", h=HPC)
        zt = const.tile([33, 512], F32, name="zt", tag="zt")
        nc.vector.memset(zt[:], 1.0)
        # select matrix for the tail 1/Z broadcast as one PE outer product
        # (rb = sel.T @ rc): row 0 -> partitions 0:64, row 32 -> 64:128.
        sel = const.tile([33, 128], F32, name="sel", tag="sel")
        nc.vector.memset(sel[:], 0.0)
        nc.vector.memset(sel[0:1, 0:64], 1.0)
        nc.vector.memset(sel[32:33, 64:128], 1.0)

        # ---------------- compute building blocks ----------------
        def warmup(n):
            # garbage matmuls: lift/hold the HAM clock gate during DMA
            # waits. Allocated from ps_a (unused while no unit is live /
            # rotation-safe) so they never sit inside an open ps_b group.
            for _ in range(n):
                ps = ps_a.tile([128, 1024], F32, name="warm", tag="psa")
                nc.tensor.matmul(ps[:, 0:512], scratch[:, 0:128],
                                 scratch[:, 0:512], start=True, stop=True)

        def proj_block(which, ft, b, pace=0, pace_all=False):
            # pace>0: interleave warmup matmuls between the DMA-gated
            # chunk matmuls so the PE never idles during the lead-in.
            wt, bt, dst = ((wk, bk_t, kcT) if which == "k" else (wq, bq_t, qcT))
            ps = ps_b.tile([128, 512], F32, name="psp", tag="psb")
            for kk in range(KC):
                if pace and (pace_all or kk < 6):
                    warmup(pace)
                nc.tensor.matmul(ps[:], wt[:, kk, ft * 128:(ft + 1) * 128],
                                 xb[(which, b, kk)],
                                 start=(kk == 0), stop=(kk == KC - 1))
            nc.vector.tensor_scalar_add(
                dst[ft][:, b * 512:(b + 1) * 512], ps[:], bt[:, ft:ft + 1])

        def v_tile(tt):
            G, j = tt // 2, tt % 2
            ps = ps_b.tile([128, 256], F32, name="psv", tag="psb")
            for kk in range(KC):
                nc.tensor.matmul(ps[:],
                                 xb[("v", tt // 4, kk)][:, (tt % 4) * 128:(tt % 4 + 1) * 128],
                                 wv[:, kk, :],
                                 start=(kk == 0), stop=(kk == KC - 1))
            nc.vector.tensor_tensor(
                v2[G][:, j, :, 0:Dh],
                ps[:].rearrange("p (h d) -> p h d", h=HPC),
                bv_bc[:, :, :],
                mybir.AluOpType.add)
            nc.gpsimd.memset(v2[G][:, j, :, Dh:Dh + 1], 1.0)

        units = [(0, 0), (0, 1), (0, 2), (0, 3), (1, 0), (1, 1), (1, 2), (1, 3)]
        pvs_tiles = {}
        etiles = {}

        def pv_chunk(u, G):
            p, blk = units[u]
            if u not in pvs_tiles:
                pvs_tiles[u] = [ps_pv.tile([65, 512], F32, name=f"pv{hh}", tag=f"pv{hh}")
                                for hh in range(2)]
            pvs = pvs_tiles[u]
            et = etiles[(u, G)][:].rearrange("p (j q) -> p j q", j=2)
            for hh in range(2):
                lh = p * 2 + hh
                rhs = et[:, :, hh * 512:(hh + 1) * 512]      # [128, 2, 512]
                if FP8_PV:
                    nc.tensor.matmul(pvs[hh][:], v2[G][:, :, lh, 0:Dh + 1], rhs,
                                     start=(G == 0), stop=(G == 7),
                                     perf_mode=mybir.MatmulPerfMode.DoubleRow,
                                     skip_group_check=True)
                else:
                    for j in range(2):
                        nc.tensor.matmul(pvs[hh][:], v2[G][:, j, lh, 0:Dh + 1],
                                         rhs[:, j, :],
                                         start=(G == 0 and j == 0),
                                         stop=(G == 7 and j == 1),
                                         skip_group_check=True)

        def chain(u, tail=False):
            """1/Z normalize for unit u's PV banks -> attnT (bf16).

            tail=True: latency-optimized variant for the last unit — pv
            copies on the (idle) scalar engine, the broadcast as one PE
            outer product instead of two serial gpsimd broadcasts.
            """
            p, blk = units[u]
            tq0 = blk * 512
            pvs = pvs_tiles[u]
            for hh in range(2):
                if tail and hh == 1:
                    nc.scalar.copy(zt[hh * 32:hh * 32 + 1, :], pvs[hh][64:65, :])
                else:
                    nc.vector.tensor_copy(zt[hh * 32:hh * 32 + 1, :], pvs[hh][64:65, :])
            pvcs = []
            for hh in range(2):
                pvc = spool.tile([64, 512], BF, name=f"pvc{hh}", tag=f"pvc{hh}")
                if tail:
                    nc.scalar.copy(pvc[:], pvs[hh][0:64, :])
                else:
                    nc.vector.tensor_copy(pvc[:], pvs[hh][0:64, :])
                pvcs.append(pvc)
            rc = spool.tile([33, 512], F32, name="rc", tag="rc")
            nc.vector.reciprocal_approx_fast(rc[:], zt[:])
            if tail:
                rbp = ps_b.tile([128, 512], F32, name="rbp", tag="psb")
                nc.tensor.matmul(rbp[:], sel[:], rc[:], start=True, stop=True)
                for hh in range(2):
                    nc.vector.tensor_tensor(
                        attnT[p][hh * 64:(hh + 1) * 64, tq0:tq0 + 512],
                        pvcs[hh][:], rbp[hh * 64:(hh + 1) * 64, :],
                        mybir.AluOpType.mult)
                return
            rc1 = spool.tile([1, 512], F32, name="rc1", tag="rc1")
            nc.vector.tensor_copy(rc1[:], rc[32:33, :])
            rcaps = [rc[0:1, :], rc1[:]]
            for hh in range(2):
                rb = spool.tile([64, 512], F32, name=f"rb{hh}", tag=f"rb{hh}")
                nc.gpsimd.partition_broadcast(rb[:], rcaps[hh])
                nc.vector.tensor_tensor(
                    attnT[p][hh * 64:(hh + 1) * 64, tq0:tq0 + 512],
                    pvcs[hh][:], rb[:], mybir.AluOpType.mult)

        wo_ot = {}

        def wo_half(tt, eb, tail=False):
            # one 512-wide half of an output row group: fine-grained PE
            # filler (~0.5us) so interposed work never starves the ACT.
            # tail=True: cast on the idle scalar engine and DMA each half
            # immediately so the drain isn't DVE-serialized.
            if eb == 0:
                ot = opool.tile([128, E], BF, name="ot", tag="ot")
                wo_ot[tt] = ot
            else:
                ot = wo_ot.pop(tt)
            ps = ps_b.tile([128, 512], F32, name="pswo", tag="psb")
            for fc in range(2):
                nc.tensor.matmul(ps[:], attnT[fc][:, tt * 128:(tt + 1) * 128],
                                 wo_t[:, fc, eb * 512:(eb + 1) * 512],
                                 start=(fc == 0), stop=(fc == 1))
            if tail:
                if eb == 0:
                    nc.scalar.copy(ot[:, eb * 512:(eb + 1) * 512], ps[:])
                else:
                    nc.vector.tensor_copy(ot[:, eb * 512:(eb + 1) * 512], ps[:])
                nc.sync.dma_start(
                    io["out"][tt * 128:(tt + 1) * 128, eb * 512:(eb + 1) * 512],
                    ot[:, eb * 512:(eb + 1) * 512])
                return
            nc.vector.tensor_copy(ot[:, eb * 512:(eb + 1) * 512], ps[:])
            if eb == 1:
                nc.sync.dma_start(io["out"][tt * 128:(tt + 1) * 128, :], ot[:])

        def wo_group(tt, tail=False):
            wo_half(tt, 0, tail)
            wo_half(tt, 1, tail)

        # ---------------- schedule ----------------
        # fillers placed by deadline vs DMA arrival (~358GB/s in issue
        # order); (ui, g) -> list of closures
        sched = {
            (0, 0): [lambda: proj_block("k", 0, 1)],
            (0, 2): [lambda: proj_block("k", 0, 2)],
            (0, 4): [lambda: proj_block("k", 0, 3)],
            (0, 6): [lambda: v_tile(0), lambda: v_tile(1)],
            (0, 7): [lambda: v_tile(2), lambda: v_tile(3), lambda: proj_block("q", 0, 1)],
            (1, 0): [lambda: v_tile(4), lambda: v_tile(5)],
            (1, 1): [lambda: v_tile(6), lambda: v_tile(7)],
            (1, 2): [lambda: v_tile(8), lambda: v_tile(9)],
            (1, 3): [lambda: v_tile(10), lambda: v_tile(11)],
            (1, 4): [lambda: v_tile(12), lambda: v_tile(13)],
            (1, 5): [lambda: v_tile(14), lambda: v_tile(15)],
            (1, 7): [lambda: proj_block("q", 0, 2)],
            (2, 1): [lambda: proj_block("k", 1, 0)],
            (2, 3): [lambda: proj_block("k", 1, 1)],
            (2, 4): [lambda: proj_block("q", 0, 3)],
            (2, 5): [lambda: proj_block("k", 1, 2)],
            (2, 7): [lambda: proj_block("k", 1, 3)],
            (3, 3): [lambda: proj_block("q", 1, 0)],
            (3, 5): [lambda: proj_block("q", 1, 1)],
            (4, 3): [lambda: proj_block("q", 1, 2)],
            (4, 5): [lambda: proj_block("q", 1, 3)],
            (5, 2): [lambda: wo_half(0, 0)],
            (5, 3): [lambda: wo_half(0, 1)],
            (5, 4): [lambda: wo_half(1, 0)],
            (5, 5): [lambda: wo_half(1, 1)],
            (5, 6): [lambda: wo_half(2, 0)],
            (5, 7): [lambda: wo_half(2, 1)],
            (6, 0): [lambda: wo_half(3, 0)],
            (6, 1): [lambda: wo_half(3, 1)],
            (6, 2): [lambda: wo_half(4, 0)],
            (6, 3): [lambda: wo_half(4, 1)],
            (6, 4): [lambda: wo_half(5, 0)],
            (6, 5): [lambda: wo_half(5, 1)],
            (6, 6): [lambda: wo_half(6, 0)],
            (6, 7): [lambda: wo_half(6, 1)],
            (7, 0): [lambda: wo_half(7, 0)],
            (7, 1): [lambda: wo_half(7, 1)],
            (7, 2): [lambda: wo_half(8, 0), lambda: wo_half(8, 1)],
            (7, 3): [lambda: wo_half(9, 0)],
            (7, 4): [lambda: wo_half(9, 1)],
            (7, 5): [lambda: wo_half(10, 0)],
            (7, 6): [lambda: wo_half(10, 1)],
            (7, 7): [lambda: wo_half(11, 0, True)],
        }

        # PV FIFO: chunk (u, G) may emit once exp (u, G+1) is emitted
        # (lag-1) and v2[G] is fully projected; chain(u) follows chunk(u,7).
        pv_q = deque()
        emitted = set()
        vcount = [0]

        def pv_ready(item, ui, g):
            kind = item[0]
            if kind == "chain":
                return True
            _, u, G = item
            if vcount[0] < 2 * (G + 1):
                return False
            need = (u, G + 1) if G < 7 else ((u + 1, 0) if u < 7 else None)
            return need is None or need in emitted

        def drain_pv(ui, g, cap=2):
            n = 0
            while pv_q and n < cap:
                item = pv_q[0]
                if not pv_ready(item, ui, g):
                    break
                pv_q.popleft()
                if item[0] == "chain":
                    chain(item[1], tail=(item[1] == 7))
                else:
                    pv_chunk(item[1], item[2])
                    n += 1

        # ---------------- emission ----------------
        warmup(8)
        proj_block("k", 0, 0, pace=2)
        proj_block("q", 0, 0, pace=2)

        orig_vtile = v_tile

        def v_tile_counted(tt):
            orig_vtile(tt)
            vcount[0] += 1

        v_tile = v_tile_counted
        # patch sched closures to use counted v_tile: rebuild lazily instead
        # (closures above captured the name `v_tile` at call time in this
        # scope, so they already see the counted version)

        for ui, (p, blk) in enumerate(units):
            for G in range(8):
                pv_q.append(("chunk", ui, G))
            pv_q.append(("chain", ui))
            for g in range(8):
                etile = epool.tile([128, 2 * 1024], act_dt, name="exp", tag="exp")
                etiles[(ui, g)] = etile
                for j2 in range(2):
                    tk = g * 2 + j2
                    ps = ps_a.tile([128, 1024], F32, name="psa", tag="psa")
                    for hh in range(2):
                        pp = hh * 64
                        nc.tensor.matmul(
                            ps[:, hh * 512:(hh + 1) * 512],
                            kcT[p][pp:pp + 64, tk * 128:(tk + 1) * 128],
                            qcT[p][pp:pp + 64, blk * 512:blk * 512 + 512],
                            start=True, stop=True)
                    nc.scalar.activation(
                        etile[:, j2 * 1024:(j2 + 1) * 1024], ps[:],
                        mybir.ActivationFunctionType.Exp, scale=1.0 / SCALE)
                emitted.add((ui, g))
                drain_pv(ui, g)
                for fn in sched.get((ui, g), ()):
                    fn()
        # tail: flush remaining PV chunks + chain; keep the PE warm with
        # dummy matmuls while the normalize chain runs, then the last Wo
        # block with DMAs split across both HW queues.
        while pv_q:
            item = pv_q.popleft()
            if item[0] == "chain":
                chain(item[1], tail=(item[1] == 7))
            else:
                pv_chunk(item[1], item[2])
        wo_half(11, 1, True)
        warmup(12)
        for tt in (12, 13, 14, 15):
            wo_group(tt, tail=True)


def _build():
    nc = bacc.Bacc("TRN2", target_bir_lowering=False, debug=False)
    io = {}
    for name, shape, dt in (
        ("xq", [128, KC, T], BF),
        ("xk", [128, KC, T], BF),
        ("xv", [128, KC, T], BF),
        ("wq", [128, KC, F], BF),
        ("wk", [128, KC, F], BF),
        ("wv", [128, KC, F], BF),
        ("wo", [128, 2, E], BF),
        ("bq", [128, 2], F32),
        ("bk", [128, 2], F32),
        ("bv", [1, F], F32),
    ):
        io[name] = nc.dram_tensor(name, shape, dt, kind="ExternalInput").ap()
    io["out"] = nc.dram_tensor("out", [T, E], BF, kind="ExternalOutput").ap()
    with tile.TileContext(nc) as tc:
        _emit(tc, io)
    nc.compile()
    return nc


def _fold_clr(W, b, clr):
    """q_c = q - mean_head(q) + clr  ==  x @ (C W).T + (C b + clr)."""
    W64 = W.astype(np.float64).reshape(H, Dh, E)
    W_eff = W64 - W64.mean(axis=1, keepdims=True)
    b64 = b.astype(np.float64).reshape(H, Dh)
    b_eff = b64 - b64.mean(axis=1, keepdims=True) + clr.astype(np.float64).reshape(H, Dh)
    return W_eff.reshape(E, E), b_eff.reshape(E)


_NC_CACHE = None


def _chunk3(a, nchunk):
    """[nchunk*128, M] -> [128, nchunk, M]"""
    n, m = a.shape
    return np.ascontiguousarray(
        a.reshape(nchunk, 128, m).transpose(1, 0, 2))


def kernel(**inputs):
    global _NC_CACHE
    query = np.asarray(inputs["query"], np.float32)
    key = np.asarray(inputs["key"], np.float32)
    value = np.asarray(inputs["value"], np.float32)
    mask = np.asarray(inputs["key_padding_mask"])
    Wq, bq = np.asarray(inputs["Wq"], np.float32), np.asarray(inputs["bq"], np.float32)
    Wk, bk = np.asarray(inputs["Wk"], np.float32), np.asarray(inputs["bk"], np.float32)
    Wv, bv = np.asarray(inputs["Wv"], np.float32), np.asarray(inputs["bv"], np.float32)
    Wo, bo = np.asarray(inputs["Wo"], np.float32), np.asarray(inputs["bo"], np.float32)
    cq = np.asarray(inputs["clr_bias_q"], np.float32)
    ck = np.asarray(inputs["clr_bias_k"], np.float32)
    assert not mask.any(), "kernel assumes empty key_padding_mask"

    Wq_eff, bq_eff = _fold_clr(Wq, bq, cq)
    Wk_eff, bk_eff = _fold_clr(Wk, bk, ck)

    def bf(x):
        return np.ascontiguousarray(x.astype(np.float32)).astype(BF_NP)

    in_maps = []
    for c in range(NCORES):
        b = c // 4
        fs = (c % 4) * F
        m = {
            "xq": bf(_chunk3(query[b].T, KC)),
            "xk": bf(_chunk3(key[b].T, KC)),
            "xv": bf(_chunk3(value[b].T, KC)),
            "wq": bf(_chunk3(Wq_eff[fs:fs + F].T, KC)),
            "wk": bf(_chunk3(Wk_eff[fs:fs + F].T, KC)),
            "wv": bf(_chunk3(Wv[fs:fs + F].T, KC)),
            "wo": bf(_chunk3(Wo[:, fs:fs + F].T, 2)),
            "bq": np.ascontiguousarray(
                bq_eff[fs:fs + F].reshape(2, 128).T.astype(np.float32)),
            "bk": np.ascontiguousarray(
                bk_eff[fs:fs + F].reshape(2, 128).T.astype(np.float32)),
            "bv": np.ascontiguousarray(bv[None, fs:fs + F], dtype=np.float32),
        }
        in_maps.append(m)

    if _NC_CACHE is None:
        _NC_CACHE = _build()
    nc = _NC_CACHE

    import os

    trace = bool(int(os.environ.get("KERNEL_TRACE", "0")))
    if trace:
        _install_ntff_hook()
    res = None
    last_exc = None
    out = None
    for attempt in range(4):
        try:
            res = run_bass_kernel_spmd(
                nc, in_maps, core_ids=list(range(NCORES)), trace=trace
            )
        except Exception as e:  # transient NRT_EXEC_UNIT_UNRECOVERABLE etc.
            last_exc = e
            import time

            time.sleep(2.0)
            continue
        out = np.zeros((B, T, E), np.float32)
        for c in range(NCORES):
            out[c // 4] += res.results[c]["out"].astype(np.float32)
        if np.isfinite(out).all():
            break
        out = None  # rare transient corruption: retry
    if out is None:
        if last_exc is not None and res is None:
            raise last_exc
        raise RuntimeError("kernel produced non-finite output on all attempts")
    kernel.last_results = res
    out += bo[None, None, :].astype(np.float32)
    return out
